# revision 1
# baseline (speedup 1.0000x reference)
"""TRN2 Bass kernel for nn_CIN (2-layer Compressed Interaction Network).

Reference computation (per sample b):
  inter0[(p,q),d] = xe[b,p,d] * xe[b,q,d]          (F=39 fields, D=16)
  x1[h,d]  = sum_{p,q} W0[h, p*39+q] * inter0[(p,q),d]   (h=128)
  out0[h]  = sum_d x1[h,d]
  inter1[(i,j),d] = x1[i,d] * xe[b,j,d]
  out1[h]  = sum_d sum_{i,j} W1[h, i*39+j] * inter1[(i,j),d]
           = sum_{i,j} W1[h,i,j] * G[i,j],  G[i,j] = sum_d x1[i,d]*xe[b,j,d]
  out = concat(out0, out1)    -> [B, 256]

Strategy (8-core data parallel, 256 samples/core):
  * Symmetrize layer 0: only 780 unique (p<=q) pairs, W0sym folds the x2.
    Pairs packed as 780 rows = 6 chunks of 128 + tail of 12, laid out as
    wrapped bands (k=0 diag, then (p,(p+k)%39) for k=1..19).
  * Host ships row-replicated A/B operands (A[m]=xT[p(m)], B[m]=xT[q(m)])
    in fp16; device does one wide DVE multiply per bd-quarter to build the
    interaction rows, then PSUM-accumulated fp16 matmuls against W0sym.
  * Layer 1 via the Gram trick: PE-transpose x1 into [(b8,d), h] blocks,
    matmul against a host-built block-diagonal xe tensor (+ a ones column
    that produces out0 for free), then 39 accumulating matmuls against
    per-field W1 slices with strided G reads.
  * fp16 everywhere on the wide paths (2x DVE mode, 1 cyc/col PE);
    fp32 PSUM accumulation and an fp32 output path.
"""

import sys

sys.path.insert(0, "/opt/trn_rl_repo")

import numpy as np
import ml_dtypes

F16 = ml_dtypes.float16 if hasattr(ml_dtypes, "float16") else np.float16

NUM_FIELD = 39
H = 128            # CIN layer width (both layers)
D = 16             # embed dim
BATCH = 2048
NCORES = 8
B_LOC = BATCH // NCORES          # 256
BD = B_LOC * D                   # 4096 columns, b-major / d-minor
NQ = 4                           # bd quarters
QCOLS = BD // NQ                 # 1024 cols = 64 samples
QSAMP = B_LOC // NQ              # 64
NPAIR = 780                      # unique (p<=q) pairs
CS = 117                         # chunk rows (3*39: A-operand periodic)
NCH = 7                          # ceil(780/117); last chunk zero-padded
NPAD = CS * NCH                  # 819
NGRP = B_LOC // 8                # 32 groups of 8 samples
GW = 40                          # 39 fields + ones column
GCOLS = NGRP * 8 * GW            # 10240


def _pairs():
    """Wrapped-band enumeration of the 780 unique pairs."""
    ps, qs = [], []
    for p in range(NUM_FIELD):           # band 0: diagonal
        ps.append(p); qs.append(p)
    for k in range(1, 20):               # bands 1..19
        for p in range(NUM_FIELD):
            ps.append(p); qs.append((p + k) % NUM_FIELD)
    return np.array(ps), np.array(qs)


_P_IDX, _Q_IDX = _pairs()

_COMPILED = None


def _build_module(debug_taps=False, reps=1):
    import concourse.bass as bass
    import concourse.bacc as bacc
    import concourse.mybir as mybir
    from concourse import tile

    f32 = mybir.dt.float32
    f16 = mybir.dt.float16

    nc = bacc.Bacc("TRN2", target_bir_lowering=False, debug=False)

    taps = {}
    if debug_taps:
        taps["x1sb"] = nc.dram_tensor("dbg_x1sb", [128, BD], mybir.dt.float16, kind="ExternalOutput")
        taps["x1tsb"] = nc.dram_tensor("dbg_x1tsb", [128, NQ, 8, 128], mybir.dt.float16, kind="ExternalOutput")
        taps["gsb"] = nc.dram_tensor("dbg_gsb", [128, GCOLS], mybir.dt.float16, kind="ExternalOutput")
        taps["out0sb"] = nc.dram_tensor("dbg_out0sb", [128, B_LOC], mybir.dt.float32, kind="ExternalOutput")
        taps["out1sb"] = nc.dram_tensor("dbg_out1sb", [128, B_LOC], mybir.dt.float32, kind="ExternalOutput")
        taps["inter0"] = nc.dram_tensor("dbg_inter0", [CS, NCH, QCOLS], mybir.dt.float16, kind="ExternalOutput")

    # ---- DRAM parameters (per-core shards prepared host-side) ----
    # B operand fully replicated; A operand is periodic = tile(xT, 3)
    B_main = nc.dram_tensor("B_main", [NQ, CS, NCH, QCOLS], f16, kind="ExternalInput")
    XT3 = nc.dram_tensor("XT3", [CS, BD], f16, kind="ExternalInput")
    CONSTA = nc.dram_tensor("CONSTA", [128, NCH * H + 128], f16, kind="ExternalInput")
    CONSTB = nc.dram_tensor("CONSTB", [128, NUM_FIELD * H], f16, kind="ExternalInput")
    BDX = nc.dram_tensor("BDX", [128, GCOLS], f16, kind="ExternalInput")
    IDT32 = nc.dram_tensor("IDT32", [128, 128], f32, kind="ExternalInput")
    out = nc.dram_tensor("out", [B_LOC, 2 * H], f32, kind="ExternalOutput")

    with tile.TileContext(nc) as tc:
        with tc.tile_pool(name="const", bufs=1) as cpool, \
             tc.tile_pool(name="ab", bufs=4) as abpool, \
             tc.tile_pool(name="inter", bufs=2) as ipool, \
             tc.tile_pool(name="x1p", bufs=2) as x1pool, \
             tc.tile_pool(name="psA", bufs=1, space="PSUM") as psA, \
             tc.tile_pool(name="psB", bufs=1, space="PSUM") as psB:

            # ---- constants / weights (single blob DMA on ACT queue) ----
            xtx3 = cpool.tile([CS, BD], f16, tag="xtx3")
            nc.gpsimd.dma_start(xtx3[:], XT3[:])
            ca = cpool.tile([128, NCH * H + 128], f16, tag="ca")
            nc.gpsimd.dma_start(ca[:], CONSTA[:])
            w0t = ca[:, 0:NCH * H].rearrange("p (c h) -> p c h", c=NCH)
            id16 = ca[:, NCH * H:NCH * H + 128]
            w1tt = cpool.tile([128, NUM_FIELD * H], f16, tag="w1tt")
            w1t = w1tt[:].rearrange("p (j h) -> p j h", j=NUM_FIELD)
            bdx = cpool.tile([128, GCOLS], f16, tag="bdx")
            id32 = cpool.tile([128, 128], f32, tag="id32")

            gsb = cpool.tile([128, GCOLS], f16, tag="gsb")          # G, fp16
            out0sb = cpool.tile([128, B_LOC], f32, tag="out0sb")     # [h, b]

            for rep in range(reps):
              out1ps = psB.tile([128, 2 * H], f32, tag="out1ps")       # 1 bank
              g_r = gsb[:].rearrange("p (b j) -> p b j", j=GW)
              x1tall = cpool.tile([128, NGRP, 128], f16, tag="x1tall")
              x1sbs = {}

              def emit_transposes(q):
                  x1tp = psA.tile([128, 8, 128], f16, tag="x1tp")
                  for t in range(8):
                      nc.tensor.transpose(
                          x1tp[:, t, :], x1sbs[q][:, t * 128:(t + 1) * 128], id16[:]
                      )
                  nc.scalar.copy(x1tall[:, q * 8:(q + 1) * 8, :], x1tp[:])
                  if debug_taps:
                      nc.sync.dma_start(taps["x1tsb"][:, q], x1tall[:, q * 8:(q + 1) * 8, :])

              def emit_g_half(half, rounds=range(8)):
                  for r in rounds:                     # 2 groups per round
                      gps = psB.tile([128, 2, 512], f32, tag=f"gps{r % 2}")
                      for gl in range(2):
                          gi = half * 16 + r * 2 + gl  # global group id
                          nc.tensor.matmul(
                              gps[:, gl, 0:GW * 8],
                              x1tall[:, gi, :],
                              bdx[:, gi * GW * 8:(gi + 1) * GW * 8],
                              start=True, stop=True,
                          )
                      gi0 = half * 16 + r * 2
                      geng = nc.scalar if (half * 8 + r) % 2 == 0 else nc.vector
                      gcopy = geng.copy if geng is nc.scalar else geng.tensor_copy
                      gcopy(
                          gsb[:, gi0 * GW * 8:(gi0 + 2) * GW * 8]
                             .rearrange("p (g n) -> p g n", g=2),
                          gps[:, :, 0:GW * 8],
                      )

              # ===== B loads: head quarters split across queues =====
              b_ts = []
              for q in range(NQ):
                  bq = abpool.tile([CS, NCH, QCOLS], f16, tag="b_t", name=f"b{q}_{rep}")
                  b_ts.append(bq)
              if rep == 0:
                  nc.sync.dma_start(b_ts[0][:, 0:4, :], B_main[0][:, 0:4, :])
                  nc.scalar.dma_start(b_ts[0][:, 4:7, :], B_main[0][:, 4:7, :])
                  nc.sync.dma_start(b_ts[1][:, 0:4, :], B_main[1][:, 0:4, :])
                  nc.gpsimd.dma_start(b_ts[1][:, 4:7, :], B_main[1][:, 4:7, :])
                  nc.sync.dma_start(b_ts[2][:], B_main[2])
                  nc.gpsimd.dma_start(b_ts[3][:], B_main[3])
                  nc.scalar.dma_start(id32[:], IDT32[:])
                  nc.scalar.dma_start(bdx[:], BDX[:])
                  nc.sync.dma_start(w1tt[:], CONSTB[:])
              else:
                  for q in range(NQ):
                      (nc.sync if q % 2 == 0 else nc.gpsimd).dma_start(
                          b_ts[q][:], B_main[q])

              # ===== pipeline =====
              for q in range(NQ):
                  b_t = b_ts[q]
                  inter = ipool.tile([CS, NCH, QCOLS], f16, tag="inter")
                  a_view = xtx3[:, q * QCOLS:(q + 1) * QCOLS] \
                      .unsqueeze(1).broadcast_to([CS, NCH, QCOLS])
                  nc.vector.tensor_mul(inter[:], a_view, b_t[:])

                  x1ps = psA.tile([128, QCOLS], f32, tag="x1ps")
                  for s in range(QCOLS // 512):
                      for c in range(NCH):
                          nc.tensor.matmul(
                              x1ps[:, s * 512:(s + 1) * 512],
                              w0t[0:CS, c, :],
                              inter[:, c, s * 512:(s + 1) * 512],
                              start=(c == 0), stop=(c == NCH - 1),
                          )

                  x1sb = x1pool.tile([128, QCOLS], f16, tag="x1sb")
                  nc.scalar.copy(x1sb[:], x1ps[:])
                  x1sbs[q] = x1sb
                  if debug_taps:
                      nc.sync.dma_start(taps["x1sb"][:, q * QCOLS:(q + 1) * QCOLS], x1sb[:])
                      if q == 0:
                          nc.sync.dma_start(taps["inter0"][:], inter[:])

                  # previous quarter's transposes AFTER this quarter's matmuls
                  if q >= 1:
                      emit_transposes(q - 1)
                  if q == 1:
                      emit_g_half(0, range(0, 4))
                  if q == 2:
                      emit_g_half(0, range(4, 8))
                  if q == 3:
                      # groups 16-23 depend only on quarter 2's transposes:
                      # run them while quarter 3's x1 bounces through ACT
                      emit_g_half(1, range(0, 4))
                      emit_transposes(3)
                      emit_g_half(1, range(4, 8))

              for hb, hn in ((0, 128), (128, 64), (192, 64)):
                  for j in range(NUM_FIELD):
                      nc.tensor.matmul(
                          out1ps[:, hb:hb + hn],
                          w1t[:, j, :],
                          g_r[:, hb:hb + hn, j],
                          start=(j == 0), stop=(j == NUM_FIELD - 1),
                      )

              out1sb = cpool.tile([128, B_LOC], f32, tag="out1sb")
              nc.scalar.copy(out1sb[:], out1ps[:, 0:B_LOC])
              if debug_taps:
                  nc.sync.dma_start(taps["gsb"][:], gsb[:])
                  nc.sync.dma_start(taps["out1sb"][:], out1sb[:])

              # ---- final transposes to [b, (out0|out1)] ----
              # out0 read from gsb ones-columns (f16)
              finp16 = psB.tile([128, 2, 128], f16, tag="gps1")
              nc.tensor.transpose(finp16[:, 0, :], g_r[:, 0:128, 39], id16[:])
              nc.tensor.transpose(finp16[:, 1, :], g_r[:, 128:256, 39], id16[:])
              finp32 = psB.tile([128, 2, 128], f32, tag="gps0")
              nc.tensor.transpose(finp32[:, 0, :], out1sb[:, 0:128], id32[:])
              nc.tensor.transpose(finp32[:, 1, :], out1sb[:, 128:256], id32[:])
              finsb = cpool.tile([128, 4, 128], f32, tag="finsb")
              fin_r = finsb[:]
              nc.scalar.copy(fin_r[:, 0::2, :], finp16[:])
              nc.scalar.copy(fin_r[:, 1::2, :], finp32[:])
              nc.sync.dma_start(
                  out[:].rearrange("(blk b) ch -> b blk ch", blk=2),
                  finsb[:].rearrange("p (a b) n -> p a (b n)", a=2),
              )

    nc.compile()
    return nc


def _host_prep(x_emb, W0, W1):
    """Build per-core input maps."""
    maps = []
    # weights: symmetrized / repacked, shared by all cores
    W0m = W0.reshape(H, NUM_FIELD, NUM_FIELD)
    W0sym = W0m[:, _P_IDX, _Q_IDX] + np.where(
        (_P_IDX != _Q_IDX)[None, :], W0m[:, _Q_IDX, _P_IDX], 0.0
    )                                            # [H, 780]
    W0p = np.zeros((H, NPAD), np.float32)
    W0p[:, :NPAIR] = W0sym
    w0t = np.zeros((128, NCH, H), np.float32)
    w0t[0:CS] = W0p.T.reshape(NCH, CS, H).transpose(1, 0, 2)
    w0t = w0t.astype(F16)

    w1t = np.ascontiguousarray(
        W1.reshape(H, H, NUM_FIELD).transpose(1, 2, 0)
    ).astype(F16)                                # [i, j, h]

    id16 = np.eye(128, dtype=F16)
    id32 = np.eye(128, dtype=np.float32)

    for core in range(NCORES):
        xe = x_emb[core * B_LOC:(core + 1) * B_LOC]          # [256, 39, 16]
        xT = np.ascontiguousarray(xe.transpose(1, 0, 2)).reshape(NUM_FIELD, BD)
        xT16 = xT.astype(F16)

        Bm = xT16[_Q_IDX]                                    # [780, 4096]
        Bp = np.zeros((NPAD, BD), F16)
        Bp[:NPAIR] = Bm
        B_main = np.ascontiguousarray(
            Bp.reshape(NCH, CS, NQ, QCOLS).transpose(2, 1, 0, 3))
        xt3 = np.tile(xT16, (3, 1))                          # [117, 4096]

        # block-diagonal xe (+ ones column), [128=(b8,d), 32grp*8b*40]
        bdx = np.zeros((128, NGRP, 8, GW), np.float32)
        xe_t = xe.transpose(0, 2, 1)                         # [b, d, j]
        for bb in range(8):
            rows = slice(bb * D, (bb + 1) * D)
            # samples with b % 8 == bb : b = g*8 + bb
            bdx[rows, :, bb, 0:NUM_FIELD] = (
                xe_t[bb::8].transpose(1, 0, 2))              # [d, g, j]
            bdx[rows, :, bb, 39] = 1.0
        bdx = bdx.reshape(128, GCOLS).astype(F16)

        consta = np.concatenate([w0t.reshape(128, -1), id16], axis=1).astype(F16)
        constb = np.ascontiguousarray(w1t.reshape(128, -1)).astype(F16)
        maps.append({
            "B_main": B_main, "XT3": xt3, "BDX": bdx,
            "CONSTA": consta, "CONSTB": constb, "IDT32": id32,
        })
    return maps


def kernel(x_emb, W0, W1, _trace=False, _trace_kwargs=None):
    global _COMPILED
    if _COMPILED is None:
        _COMPILED = _build_module()
    nc = _COMPILED

    from concourse.bass_utils import run_bass_kernel_spmd

    in_maps = _host_prep(np.asarray(x_emb, np.float32),
                         np.asarray(W0, np.float32),
                         np.asarray(W1, np.float32))
    kw = {}
    if _trace:
        kw["trace"] = True
        kw.update(_trace_kwargs or {})
    res = run_bass_kernel_spmd(nc, in_maps, list(range(NCORES)), **kw)
    outp = np.concatenate([res.results[i]["out"] for i in range(NCORES)], axis=0)
    if _trace:
        return outp.astype(np.float32), res
    return outp.astype(np.float32)



# revision 11
# speedup vs baseline: 1.1855x; 1.1855x over previous
"""TRN2 Bass kernel for nn_CIN (2-layer Compressed Interaction Network).

Reference computation (per sample b):
  inter0[(p,q),d] = xe[b,p,d] * xe[b,q,d]          (F=39 fields, D=16)
  x1[h,d]  = sum_{p,q} W0[h, p*39+q] * inter0[(p,q),d]   (h=128)
  out0[h]  = sum_d x1[h,d]
  out1[h]  = sum_{i,j} W1[h,i,j] * G[i,j],  G[i,j] = sum_d x1[i,d]*xe[b,j,d]
  out = concat(out0, out1)    -> [B, 256]

Strategy (8-core data parallel, 256 samples/core), v2:
  * Symmetrize layer 0: 780 unique (p<=q) pairs packed as wrapped bands,
    7 chunks of 117 rows. Host ships the row-replicated B operand (fp16)
    plus XT3 = tile(xT, 3); one wide DVE multiply per chunk-group builds
    the interaction rows.
  * Transposed layer-0 matmuls: stationary = 128-column interaction
    blocks (data), moving = W0 chunk -> x1 lands PRE-TRANSPOSED as
    [(sample8, d), h] PSUM blocks. No PE transposes, single f32->f16
    drain into x1tall.
  * Layer 1 via quadrant G matmuls: per 8-sample group, 4 matmuls of
    contract 32 (sample pairs, tile_position quadrants) against a
    compact block-diagonal-2 xe tensor (bd2, 1/4 the bytes of the full
    block-diagonal form). A ones column yields out0 for free.
  * out1 via transposed chains: stationary = G half-batch slice (data),
    moving = W1 per field -> out1 lands as [b, h], no final transposes.
  * PE warm-up matmuls burn the p-state ramp while the first DMAs land.
  * DMA queues: SP + Pool stream the B operand interleaved per quarter;
    ACT ships the small constants early, then handles drains.
"""

import sys

sys.path.insert(0, "/opt/trn_rl_repo")

import numpy as np

F16 = np.float16

NUM_FIELD = 39
H = 128            # CIN layer width (both layers)
D = 16             # embed dim
BATCH = 2048
NCORES = 8
B_LOC = BATCH // NCORES          # 256
BD = B_LOC * D                   # 4096 columns, b-major / d-minor
NQ = 4                           # bd quarters
QCOLS = BD // NQ                 # 1024 cols = 64 samples
NPAIR = 780                      # unique (p<=q) pairs
CS = 117                         # chunk rows (3*39: A-operand periodic)
NCH = 7                          # ceil(780/117); last chunk zero-padded
NPAD = CS * NCH                  # 819
NGRP = B_LOC // 8                # 32 groups of 8 samples
GW = 40                          # 39 fields + ones column
N_WARM = 6                       # PE warm-up matmuls (p-state ramp)


def _pairs():
    """Wrapped-band enumeration of the 780 unique pairs."""
    ps, qs = [], []
    for p in range(NUM_FIELD):           # band 0: diagonal
        ps.append(p); qs.append(p)
    for k in range(1, 20):               # bands 1..19
        for p in range(NUM_FIELD):
            ps.append(p); qs.append((p + k) % NUM_FIELD)
    return np.array(ps), np.array(qs)


_P_IDX, _Q_IDX = _pairs()

_COMPILED = None


def _build_module(debug_taps=False, reps=1):
    import concourse.bass as bass
    import concourse.bacc as bacc
    import concourse.mybir as mybir
    from concourse import tile

    f32 = mybir.dt.float32
    f16 = mybir.dt.float16

    nc = bacc.Bacc("TRN2", target_bir_lowering=False, debug=False)

    # ---- DRAM parameters (per-core shards prepared host-side) ----
    B_main = nc.dram_tensor("B_main", [NQ, CS, NCH, QCOLS], f16, kind="ExternalInput")
    XT3 = nc.dram_tensor("XT3", [CS, BD], f16, kind="ExternalInput")
    CONSTA = nc.dram_tensor("CONSTA", [128, NCH * H + 128], f16, kind="ExternalInput")
    CONSTB = nc.dram_tensor("CONSTB", [128, NUM_FIELD * H], f16, kind="ExternalInput")
    BD2 = nc.dram_tensor("BD2", [128, NGRP, 2, GW], f16, kind="ExternalInput")
    out = nc.dram_tensor("out", [B_LOC, 2 * H], f32, kind="ExternalOutput")

    with tile.TileContext(nc) as tc:
        with tc.tile_pool(name="const", bufs=1) as cpool, \
             tc.tile_pool(name="ab", bufs=4) as abpool, \
             tc.tile_pool(name="inter", bufs=2) as ipool, \
             tc.tile_pool(name="psX", bufs=2, space="PSUM") as psX, \
             tc.tile_pool(name="psG", bufs=2, space="PSUM") as psG, \
             tc.tile_pool(name="psO", bufs=1, space="PSUM") as psO:

            # ---- constants ----
            ca = cpool.tile([128, NCH * H + 128], f16, tag="ca")
            nc.scalar.dma_start(ca[:], CONSTA[:])
            w0t = ca[:, 0:NCH * H].rearrange("p (c h) -> p c h", c=NCH)
            id16 = ca[:, NCH * H:NCH * H + 128]
            w1tt = cpool.tile([128, NUM_FIELD * H], f16, tag="w1tt")
            nc.scalar.dma_start(w1tt[:], CONSTB[:])
            w1t = w1tt[:].rearrange("p (j h) -> p j h", j=NUM_FIELD)
            bd2 = cpool.tile([128, NGRP, 2, GW], f16, tag="bd2")

            xtx3 = cpool.tile([CS, BD], f16, tag="xtx3")
            nc.sync.dma_start(xtx3[:], XT3[:])

            # B operand: SP carries chunks 4:7 (+bd2), Pool chunks 0:4
            b_ts = []
            for q in range(NQ):
                bq = abpool.tile([CS, NCH, QCOLS], f16, tag="b_t", name=f"b{q}")
                nc.gpsimd.dma_start(bq[:, 0:4, :], B_main[q][:, 0:4, :])
                nc.sync.dma_start(bq[:, 4:7, :], B_main[q][:, 4:7, :])
                b_ts.append(bq)
                if q == 0:
                    nc.sync.dma_start(bd2[:], BD2[:])

            # ---- PE warm-up: burn the p-state ramp on the const tile ----
            warmps = psX.tile([128, 4, H], f32, tag="x1ps")
            wview = warmps[:].rearrange("p a n -> p (a n)")
            for w in range(N_WARM):
                nc.tensor.matmul(wview, id16[:], ca[:, 0:512],
                                 start=True, stop=True)

            # ---- persistent intermediates ----
            x1tall = cpool.tile([128, NGRP, H], f16, tag="x1tall")
            gsb = cpool.tile([128, NGRP, 8, GW], f16, tag="gsb")
            finsb = cpool.tile([128, 2, 2, 128], f32, tag="finsb")

            def emit_out1_half(h):
                """Transposed out1 chain for sample half h (128 samples)."""
                o1 = psO.tile([128, H], f32, tag="o1ps")
                gslice = gsb[:, 16 * h:16 * (h + 1), :, :]
                for j in range(NUM_FIELD):
                    nc.tensor.matmul(
                        o1[:],
                        gslice[:, :, :, j],          # [128 i, (16 g, 8 s)]
                        w1t[:, j, :],                # [128 i, 128 h]
                        start=(j == 0), stop=(j == NUM_FIELD - 1),
                    )
                # out0 for this half: transpose of the gsb ones-columns
                fin0 = psO.tile([128, 128], f16, tag="fin0ps")
                nc.tensor.transpose(fin0[:], gslice[:, :, :, GW - 1], id16[:])
                nc.scalar.copy(finsb[:, h, 0, :], fin0[:])
                nc.scalar.copy(finsb[:, h, 1, :], o1[:])
                nc.gpsimd.dma_start(
                    out[128 * h:128 * (h + 1), :],
                    finsb[:, h].rearrange("p a n -> p (a n)"),
                )

            # ---- main pipeline ----
            for q in range(NQ):
                b_t = b_ts[q]
                inter = ipool.tile([CS, NCH, QCOLS], f16, tag="inter")
                xq = xtx3[:, q * QCOLS:(q + 1) * QCOLS]
                if q < NQ - 1:
                    splits = [(0, 4), (4, 7)]
                else:
                    # finer ops on the last quarter: PE unblocks per chunk
                    splits = [(0, 2), (2, 4), (4, 5), (5, 6), (6, 7)]
                for lo, hi in splits:
                    av = xq.unsqueeze(1).broadcast_to([CS, hi - lo, QCOLS])
                    nc.vector.tensor_mul(inter[:, lo:hi, :], av, b_t[:, lo:hi, :])

                for gp in range(2):              # two 4-group sets / quarter
                    gi0 = q * 8 + gp * 4
                    x1 = psX.tile([128, 4, H], f32, tag="x1ps")
                    for gl in range(4):
                        blk = gp * 4 + gl
                        st = inter[:].rearrange("p c (k n) -> p c k n", n=128)
                        for c in range(NCH):
                            nc.tensor.matmul(
                                x1[:, gl, :],
                                st[:, c, blk, :],      # stationary: data block
                                w0t[0:CS, c, :],       # moving: W0 chunk
                                start=(c == 0), stop=(c == NCH - 1),
                            )
                    nc.scalar.copy(x1tall[:, gi0:gi0 + 4, :], x1[:])

                    for gg in range(2):          # G for 2 groups at a time
                        # bank-padded: each group's [8, 40] leads a PSUM bank
                        gps = psG.tile([128, 2, 512], f32, tag="gps")
                        for gl2 in range(2):
                            gi = gi0 + gg * 2 + gl2
                            for sp in range(4):
                                nc.tensor.matmul(
                                    gps[:, gl2, 80 * sp:80 * sp + 80],
                                    x1tall[32 * sp:32 * sp + 32, gi, :],
                                    bd2[32 * sp:32 * sp + 32, gi, :, :],
                                    start=True, stop=True,
                                    tile_position=(32 * sp, 0),
                                )
                        gi = gi0 + gg * 2
                        gview = gps[:, :, 0:8 * GW].rearrange(
                            "p a (s j) -> p a s j", j=GW)
                        if q < NQ - 1:
                            nc.scalar.copy(gsb[:, gi:gi + 2, :, :], gview)
                        else:
                            nc.vector.tensor_copy(gsb[:, gi:gi + 2, :, :], gview)

                if q == 1:
                    emit_out1_half(0)
            emit_out1_half(1)

    nc.compile()
    return nc


def _host_prep(x_emb, W0, W1):
    """Build per-core input maps."""
    maps = []
    # weights: symmetrized / repacked, shared by all cores
    W0m = W0.reshape(H, NUM_FIELD, NUM_FIELD)
    W0sym = W0m[:, _P_IDX, _Q_IDX] + np.where(
        (_P_IDX != _Q_IDX)[None, :], W0m[:, _Q_IDX, _P_IDX], 0.0
    )                                            # [H, 780]
    W0p = np.zeros((H, NPAD), np.float32)
    W0p[:, :NPAIR] = W0sym
    w0t = np.zeros((128, NCH, H), np.float32)
    w0t[0:CS] = W0p.T.reshape(NCH, CS, H).transpose(1, 0, 2)
    w0t = w0t.astype(F16)

    w1t = np.ascontiguousarray(
        W1.reshape(H, H, NUM_FIELD).transpose(1, 2, 0)
    ).astype(F16)                                # [i, j, h]

    id16 = np.eye(128, dtype=F16)
    consta = np.concatenate([w0t.reshape(128, -1), id16], axis=1).astype(F16)
    constb = np.ascontiguousarray(w1t.reshape(128, -1)).astype(F16)

    for core in range(NCORES):
        xe = x_emb[core * B_LOC:(core + 1) * B_LOC]          # [256, 39, 16]
        xT = np.ascontiguousarray(xe.transpose(1, 0, 2)).reshape(NUM_FIELD, BD)
        xT16 = xT.astype(F16)

        Bm = xT16[_Q_IDX]                                    # [780, 4096]
        Bp = np.zeros((NPAD, BD), F16)
        Bp[:NPAIR] = Bm
        B_main = np.ascontiguousarray(
            Bp.reshape(NCH, CS, NQ, QCOLS).transpose(2, 1, 0, 3))
        xt3 = np.tile(xT16, (3, 1))                          # [117, 4096]

        # block-diagonal-2 xe (+ ones column): [128=(s,d), g, s%2, 40]
        bd2 = np.zeros((128, NGRP, 2, GW), np.float32)
        xe_t = xe.transpose(0, 2, 1)                         # [b, d, j]
        for s in range(8):
            rows = slice(s * D, (s + 1) * D)
            s2 = s % 2
            # samples b = g*8 + s
            bd2[rows, :, s2, 0:NUM_FIELD] = xe_t[s::8].transpose(1, 0, 2)
            bd2[rows, :, s2, GW - 1] = 1.0
        bd2 = bd2.astype(F16)

        maps.append({
            "B_main": B_main, "XT3": xt3, "BD2": bd2,
            "CONSTA": consta, "CONSTB": constb,
        })
    return maps


def kernel(x_emb, W0, W1, _trace=False, _trace_kwargs=None):
    global _COMPILED
    if _COMPILED is None:
        _COMPILED = _build_module()
    nc = _COMPILED

    from concourse.bass_utils import run_bass_kernel_spmd

    in_maps = _host_prep(np.asarray(x_emb, np.float32),
                         np.asarray(W0, np.float32),
                         np.asarray(W1, np.float32))
    kw = {}
    if _trace:
        kw["trace"] = True
        kw.update(_trace_kwargs or {})
    res = run_bass_kernel_spmd(nc, in_maps, list(range(NCORES)), **kw)
    outp = np.concatenate([res.results[i]["out"] for i in range(NCORES)], axis=0)
    if _trace:
        return outp.astype(np.float32), res
    return outp.astype(np.float32)
